# revision 32
# baseline (speedup 1.0000x reference)
"""Masked multi-head attention block (B=4, N=1024, D=1024, H=16, DH=64) on 8
Trainium2 NeuronCores.

Sharding: core (b, g) = 2*b + g handles batch b and head-group g (8 of 16
heads). Each core computes qkv projections for its heads, attention, and its
partial output projection; the host sums the two head-group partials per batch.

Mask handling: the host gathers only the valid tokens per batch (padded to a
multiple of 16 with key-bias -30000 on the pad), so the device computes dense
unmasked attention over ~half the sequence; invalid token rows of the output
are b_out.

Device layout (per core, V = padded valid token count, 16-granular):
  x    [128, 8, V] bf16    gathered tokens, transposed, k-chunked (host prep)
  wqk  m-major bf16 blocks so pair-0 projections unblock after one block
  qkT  [128, V] bf16 tiles per 128-feature chunk (Q scaled by DH^-0.5)
  V'   [128, 8, 65] bf16 per key chunk (64 values + ones column for denom)
  S^T  [kh, 2, W] psum per (pair, key-chunk, query-half); one ACT exp per
       chunk covers both heads, with per-partition pad bias -> P^T bf16
  O^T  [65, W] psum accumulated over key chunks (row 64 = denominator)
  1/d  ACT ln on the denom row, PE ones-broadcast of -ln d, ACT exp
  A^T  [64, W] bf16 = O^T * (1/d); odd heads DMA-shifted to partitions 64-127
  y    [V, 1024] bf16 = sum_j Apair_j @ wopair_j, interleaved per query half
"""
import json
import os
import sys

import numpy as np

sys.path.insert(0, "/opt/trn_rl_repo")

import concourse.bass as bass
import concourse.mybir as mybir
from concourse.tile import TileContext
from concourse import bass_utils

F32 = mybir.dt.float32
F32R = mybir.dt.float32r
BF16 = mybir.dt.bfloat16
AF = mybir.ActivationFunctionType

B, N, D, H, DH = 4, 1024, 1024, 16, 64
NCORES = 8
PAD_BIAS = -30000.0


def _install_patches():
    """The walrus build in this container accepts only one semaphore wait per
    instruction; hoist extra waits onto same-engine NoOps in the BIR json."""
    if getattr(bass.Bass, "_split_waits_patched", False):
        return
    orig = bass.Bass.to_json_bytes

    def to_json_bytes_split(self, *a, **k):
        j = json.loads(orig(self, *a, **k))
        for fn in j.get("functions", []):
            for bb in fn.get("blocks", []):
                out = []
                for ins in bb.get("instructions", []):
                    si = ins.get("sync_info") or {}
                    waits = si.get("on_wait") or []
                    if len(waits) > 1:
                        for i, w in enumerate(waits[:-1]):
                            out.append({
                                "debug": ins.get("debug", 0),
                                "engine": ins["engine"],
                                "ins": [],
                                "name": f"{ins['name']}_sw{i}",
                                "opcode": "NoOp",
                                "outs": [],
                                "text_hint": "splitw",
                                "sync_info": {"on_update": [], "on_wait": [w]},
                            })
                        si["on_wait"] = [waits[-1]]
                    out.append(ins)
                bb["instructions"] = out
        return json.dumps(j).encode()

    bass.Bass.to_json_bytes = to_json_bytes_split

    def _drain_and_barrier(self, tick_clock, wait_clock):
        import re as _re
        import bass_rust as _br
        from concourse.vector_clock import ScopedClock as _SC
        gc = tick_clock.global_clock
        comps = eval(_re.match(r"VectorClock\((\[.*\])\)", repr(gc)).group(1))
        for i, v in enumerate(comps):
            if v <= 0:
                continue
            sub = [0] * len(comps)
            sub[i] = v
            nop = self.nc.sync.nop(nofuse=True, hint="final_wait")
            wait_clock.add_sem_waits(nop.ins, _SC({None: _br.VectorClock(sub)}))
        self.nc.sync.drain()
        self.nc.all_engine_barrier()
        assert self.sems is not None
        popped = self.nc._tile_sem_poison_stack.pop()
        assert popped is self._sem_poison
        self.nc.clear_and_free_semaphores(list(self.sems.allocated().values()))

    TileContext._drain_and_barrier = _drain_and_barrier

    if os.environ.get("BASSK_LDWOPT"):
        _orig_run = bass_utils.run_command

        def run_command_ldw(cmd, **kw):
            cmd = ["--enable-ldw-opt=true" if c == "--enable-ldw-opt=false"
                   else c for c in cmd]
            return _orig_run(cmd, **kw)

        bass_utils.run_command = run_command_ldw

    bass.Bass._split_waits_patched = True


def _halves(v):
    """Split width v into <=512 pieces, 128-aligned first piece. Balanced
    halves keep the S->exp->O pipeline fine-grained; a (512, rest) split
    was measured 20% slower (coarse ACT ops starve the PE)."""
    if v <= 512:
        return [(0, v)]
    w0 = 128 * (v // 2 // 128)
    return [(0, w0), (w0, v - w0)]


def _build_program(V):
    KC = -(-V // 128)          # key chunks, last chunk height K4
    K4 = V - 128 * (KC - 1)
    KH = [128] * (KC - 1) + [K4]
    QH = _halves(V)            # query halves for stage 2/3 interleave
    # stage-3 query chunks of <=128 rows, grouped per half
    CHQ = []
    for (n0, nw) in QH:
        chs = []
        q = n0
        while q < n0 + nw:
            mw = min(128, n0 + nw - q)
            chs.append((q, mw))
            q += mw
        CHQ.append(chs)

    nc = bass.Bass(trn_type="TRN2", target_bir_lowering=False, debug=False,
                   num_devices=NCORES)
    xt = nc.declare_dram_parameter("xt", [128, 8 * V], BF16, isOutput=False).ap()
    wqk = nc.declare_dram_parameter("wqk", [128, 8192], BF16, isOutput=False).ap()
    wv = nc.declare_dram_parameter("wv", [128, 4096], BF16, isOutput=False).ap()
    wo = nc.declare_dram_parameter("wo", [128, 4096], BF16, isOutput=False).ap()
    biasv = nc.declare_dram_parameter("biasv", [128, KC], F32, isOutput=False).ap()
    onesd = nc.declare_dram_parameter("onesd", [128, 72], F32R, isOutput=False).ap()
    y = nc.declare_dram_parameter("y", [V, D], BF16, isOutput=True).ap()

    with TileContext(nc) as tc:
        with tc.tile_pool(name="consts", bufs=1) as consts, \
             tc.tile_pool(name="xw", bufs=1) as xw, \
             tc.tile_pool(name="qk", bufs=1) as qkpool, \
             tc.tile_pool(name="vp", bufs=1) as vppool, \
             tc.tile_pool(name="pt", bufs=2 * KC + 2) as ptpool, \
             tc.tile_pool(name="osb", bufs=4) as opool, \
             tc.tile_pool(name="rln", bufs=2) as lnpool, \
             tc.tile_pool(name="at", bufs=1) as atpool, \
             tc.tile_pool(name="odd", bufs=2) as oddpool, \
             tc.tile_pool(name="ysb", bufs=2) as ypool, \
             tc.tile_pool(name="gen", bufs=2, space="PSUM") as genps, \
             tc.tile_pool(name="st", bufs=2, space="PSUM") as stps, \
             tc.tile_pool(name="ot", bufs=2, space="PSUM") as otps:

            bias_sb = consts.tile([128, KC], F32)
            ones_sb = consts.tile([128, 72], BF16)
            onesr_sb = consts.tile([128, 72], F32R)
            # HAM warm-up: dependency-free matmuls on an uninitialized tile
            # run during the start barrier + input-DMA window, so the PE
            # clock gate is already at 8/8 when real work arrives. Results
            # go to a scratch psum slot and are discarded.
            # (The spin tile lives in the ot pool, which is idle until the
            # first O^T accumulation ~25us in — the gen pool slots must stay
            # free for the stage-1 projection pipeline.)
            wsrc = consts.tile([128, 512], BF16, name="wsrc")
            nc.vector.memset(wsrc[:], 0.0)
            # two alternating targets: no WAW chain between consecutive
            # spins, so issue stays dense enough to trip the HAM busy window
            wpa = otps.tile([65, 512], F32, tag="ot", name="warm_a")
            wpb = otps.tile([65, 512], F32, tag="ot", name="warm_b")
            for i in range(24):
                nc.tensor.matmul((wpa if i % 2 else wpb)[0:64, 0:256],
                                 lhsT=wsrc[:, 0:64],
                                 rhs=wsrc[:, 0:256], start=True, stop=True)
            nc.sync.dma_start(out=bias_sb[:], in_=biasv[:])
            nc.sync.dma_start(out=onesr_sb[:], in_=onesd[:])
            nc.vector.tensor_copy(out=ones_sb[:], in_=onesr_sb[:])

            # ---- input loads ----
            # wqk is m-major: block m holds lhsT chunks for all k. Pair-0
            # needs blocks 0 and 4 plus ALL of x, so x is split across both
            # HWDGE rings and m0/m4 go first on sync: everything pair-0
            # needs has landed by ~8us, just as the warm-up spin ends.
            wqk_sb = [None] * 8
            xsb = [None] * 8

            def load_wqk(m):
                wt = xw.tile([128, 1024], BF16, tag=f"wqk{m}", name=f"wqk_{m}")
                nc.sync.dma_start(out=wt[:],
                                  in_=wqk[:, m * 1024:(m + 1) * 1024])
                wqk_sb[m] = wt

            def load_x(k, eng):
                t = xw.tile([128, V], BF16, tag=f"x{k}", name=f"x_{k}")
                eng.dma_start(out=t[:], in_=xt[:, k * V:(k + 1) * V])
                xsb[k] = t

            for k in (0, 2, 4, 6, 7):
                load_x(k, nc.scalar)
            load_wqk(0)
            load_wqk(4)
            for k in (1, 3, 5):
                load_x(k, nc.sync)
            for m in (1, 5, 2, 6, 3, 7):
                load_wqk(m)
            wv_sb = []
            for k in range(8):
                vt = xw.tile([128, 512], BF16, tag=f"wv{k}", name=f"wv_{k}")
                nc.gpsimd.dma_start(out=vt[:],
                                    in_=wv[:, k * 512:(k + 1) * 512])
                wv_sb.append(vt)
            wo_sb = []
            for j in range(4):
                wt = xw.tile([128, 1024], BF16, tag=f"wo{j}", name=f"wo_{j}")
                nc.gpsimd.dma_start(out=wt[:],
                                    in_=wo[:, j * 1024:(j + 1) * 1024])
                wo_sb.append(wt)

            qk_sb = [None] * 8   # m 0-3: Q^T (scaled), 4-7: K^T
            vp_sb = []           # per key chunk [128, 8, 65] bf16

            def project(m, halves=None):
                """qkT[m] = (wqk block m).T @ x, cast to bf16."""
                if qk_sb[m] is None:
                    qk_sb[m] = qkpool.tile([128, V], BF16, tag=f"qk{m}",
                                           name=f"qk_{m}")
                qt = qk_sb[m]
                for (n0, nw) in (halves if halves is not None else QH):
                    ps = genps.tile([128, 512], F32, tag="gen",
                                    name=f"qkp_{m}_{n0}")
                    for k in range(8):
                        nc.tensor.matmul(
                            ps[:, 0:nw],
                            lhsT=wqk_sb[m][:, k * 128:(k + 1) * 128],
                            rhs=xsb[k][:, n0:n0 + nw],
                            start=(k == 0), stop=(k == 7))
                    nc.vector.tensor_copy(out=qt[:, n0:n0 + nw],
                                          in_=ps[:, 0:nw])

            def vprime():
                for c in range(KC):
                    kh = KH[c]
                    ps = genps.tile([128, 512], F32, tag="gen",
                                    name=f"vpp_{c}")
                    for k in range(8):
                        nc.tensor.matmul(ps[0:kh, :],
                                         lhsT=xsb[k][:, c * 128:c * 128 + kh],
                                         rhs=wv_sb[k][:],
                                         start=(k == 0), stop=(k == 7))
                    vt = vppool.tile([128, 8, 65], BF16, tag=f"vp{c}",
                                     name=f"vt_{c}")
                    nc.vector.tensor_copy(
                        out=vt[0:kh, :, 0:64],
                        in_=ps[0:kh, :].rearrange("p (h d) -> p h d", h=8))
                    nc.vector.tensor_copy(
                        out=vt[0:kh, :, 64:65],
                        in_=ones_sb[0:kh, 0:8].rearrange(
                            "p (a b) -> p a b", b=1))
                    vp_sb.append(vt)

            def attention_s(hp, n0, nw):
                """S -> exp for head pair hp, queries [n0, n0+nw). Emitted
                separately so pair-0's exp chain can start before V' (which
                S does not depend on) occupies the static PE order."""
                qt = qk_sb[hp]
                kt = qk_sb[4 + hp]
                pts = []
                for c in range(KC):
                    kh = KH[c]
                    st = stps.tile([128, 2, 512], F32, tag="st",
                                   name=f"st_{hp}_{c}_{n0}")
                    for sub in (0, 1):
                        lo = sub * 64
                        nc.tensor.matmul(
                            st[0:kh, sub, 0:nw],
                            lhsT=kt[lo:lo + 64, c * 128:c * 128 + kh],
                            rhs=qt[lo:lo + 64, n0:n0 + nw],
                            start=True, stop=True)
                    pt = ptpool.tile([128, 2, 512], BF16, tag="pt",
                                     name=f"pt_{hp}_{c}_{n0}")
                    nc.scalar.activation(
                        out=pt[0:kh, :, 0:nw], in_=st[0:kh, :, 0:nw],
                        func=AF.Exp, bias=bias_sb[0:kh, c:c + 1],
                        scale=1.0)
                    pts.append(pt)
                return pts

            def attention_o(hp, n0, nw, pts):
                """O -> normalize for head pair hp, queries [n0, n0+nw)."""
                osb = opool.tile([65, 2, 512], BF16, tag="osb")
                for sub in (0, 1):
                    h = 2 * hp + sub
                    ot = otps.tile([65, 512], F32, tag="ot",
                                   name=f"ot_{h}_{n0}")
                    for c in range(KC):
                        kh = KH[c]
                        nc.tensor.matmul(ot[:, 0:nw],
                                         lhsT=vp_sb[c][0:kh, h, :],
                                         rhs=pts[c][0:kh, sub, 0:nw],
                                         start=(c == 0), stop=(c == KC - 1))
                    nc.vector.tensor_copy(out=osb[:, sub, 0:nw],
                                          in_=ot[:, 0:nw])
                # 1/denom for both heads: ln on the denom rows, exp(-x), then
                # PE ones-broadcast of the reciprocal (same ACT table set as
                # the softmax exp)
                rln = lnpool.tile([65, 2, 512], F32R, tag="rln")
                nc.scalar.activation(out=rln[64:65, :, 0:nw],
                                     in_=osb[64:65, :, 0:nw], func=AF.Ln)
                rex = lnpool.tile([65, 2, 512], F32R, tag="rex", name="rex")
                nc.scalar.activation(out=rex[64:65, :, 0:nw],
                                     in_=rln[64:65, :, 0:nw],
                                     func=AF.Exp, scale=-1.0)
                for sub in (0, 1):
                    rb = otps.tile([65, 512], F32, tag="ot",
                                   name=f"rb_{hp}_{sub}_{n0}")
                    nc.tensor.matmul(rb[0:64, 0:nw],
                                     lhsT=onesr_sb[64:65, 8:72],
                                     rhs=rex[64:65, sub, 0:nw],
                                     start=True, stop=True)
                    if sub == 0:
                        nc.vector.tensor_mul(at2[hp][0:64, n0:n0 + nw],
                                             osb[0:64, 0, 0:nw],
                                             rb[0:64, 0:nw])
                    else:
                        tmp = oddpool.tile([64, 512], BF16, tag="odd")
                        nc.vector.tensor_mul(tmp[:, 0:nw],
                                             osb[0:64, 1, 0:nw],
                                             rb[0:64, 0:nw])
                        nc.sync.dma_start(out=at2[hp][64:128, n0:n0 + nw],
                                          in_=tmp[:, 0:nw])

            def attention(hp, n0, nw):
                attention_o(hp, n0, nw, attention_s(hp, n0, nw))

            def outchunk(q0, mw):
                ysb = ypool.tile([128, 1024], BF16, tag="ysb")
                for c0 in (0, 512):
                    yp = genps.tile([128, 512], F32, tag="gen",
                                    name=f"yp_{q0}_{c0}")
                    for j in range(4):
                        nc.tensor.matmul(
                            yp[0:mw, :],
                            lhsT=at2[j][:, q0:q0 + mw],
                            rhs=wo_sb[j][:, c0:c0 + 512],
                            start=(j == 0), stop=(j == 3))
                    nc.vector.tensor_copy(out=ysb[0:mw, c0:c0 + 512],
                                          in_=yp[0:mw, :])
                nc.sync.dma_start(out=y[q0:q0 + mw, :], in_=ysb[0:mw, :])

            def outproj(half):
                for (q0, mw) in CHQ[half]:
                    outchunk(q0, mw)

            at2 = [atpool.tile([128, V], BF16, tag=f"at{j}", name=f"at2_{j}")
                   for j in range(4)]

            # ---- emission: pair-major, both query halves per pair, so the
            # not-yet-emitted pairs' projections are dependency-free PE
            # filler while each pair's ACT chain trickles ----
            for p in range(4):
                project(p, halves=(QH[:1] if p == 0 else None))  # Q pair p
                project(4 + p)   # K pair p
                if p == 0:
                    # pair-0 S/exp goes ahead of V' in the static order: S
                    # only needs Q-half0 + K, and this starts the ACT exp
                    # stream ~25us earlier while V' and the deferred Q-half1
                    # group fill the PE behind it
                    pts0 = attention_s(0, *QH[0])
                    if len(QH) > 1:
                        project(0, halves=QH[1:])
                    vprime()
                    attention_o(0, *QH[0], pts0)
                else:
                    attention(p, *QH[0])
                if len(QH) > 1:
                    if p == 3:
                        outproj(0)   # fills the PE during the last chain
                    attention(p, *QH[1])
            if len(QH) > 1:
                outproj(1)
            else:
                outproj(0)
    return nc


def kernel(x, mask, w_qkv, w_out, b_out):
    _install_patches()
    from concourse.bass_utils import run_bass_kernel_spmd
    import ml_dtypes
    bf16 = ml_dtypes.bfloat16

    x = np.asarray(x, dtype=np.float32)
    mask = np.asarray(mask, dtype=np.float32)
    w_qkv = np.asarray(w_qkv, dtype=np.float32)
    w_out = np.asarray(w_out, dtype=np.float32)
    b_out = np.asarray(b_out, dtype=np.float32)

    idx = [np.nonzero(mask[b] != 0.0)[0] for b in range(B)]
    nv = [len(i) for i in idx]
    if max(nv) == 0:
        return np.broadcast_to(b_out, (B, N, D)).astype(np.float32).copy()
    V = max(128, int(-(-max(nv) // 16)) * 16)
    KC = -(-V // 128)

    scale = float(DH) ** -0.5
    G = 512  # features per head-group
    wqk_g, wv_g, wo_g = [], [], []
    for g in range(2):
        wq = w_qkv[:, g * G:(g + 1) * G] * scale
        wk = w_qkv[:, 1024 + g * G:1024 + (g + 1) * G]
        wqk = np.concatenate([wq, wk], axis=1)  # [1024, 1024]
        # m-major: [128, m, k, 128]: [p, m*1024 + k*128 + f] = wqk[k*128+p,
        # m*128+f]
        wqk_g.append(np.ascontiguousarray(
            wqk.reshape(8, 128, 8, 128).transpose(1, 2, 0, 3).reshape(128, 8192)
        ).astype(bf16))
        wvg = w_qkv[:, 2048 + g * G:2048 + (g + 1) * G]  # [1024, 512]
        wv_g.append(np.ascontiguousarray(
            wvg.reshape(8, 128, 512).transpose(1, 0, 2).reshape(128, 4096)
        ).astype(bf16))
        wog = w_out[g * G:(g + 1) * G, :]  # [512, 1024]
        wo_g.append(np.ascontiguousarray(
            wog.reshape(4, 128, 1024).transpose(1, 0, 2).reshape(128, 4096)
        ).astype(bf16))

    xt_b, bias_b = [], []
    for b in range(B):
        pad = V - nv[b]
        idxp = np.concatenate([idx[b], np.zeros(pad, dtype=np.int64)])
        xg = x[b][idxp, :]  # [V, 1024]
        xt_b.append(np.ascontiguousarray(
            xg.T.reshape(8, 128, V).transpose(1, 0, 2).reshape(128, 8 * V)
        ).astype(bf16))
        bv = np.concatenate([
            np.zeros(nv[b], dtype=np.float32),
            np.full(pad, PAD_BIAS, dtype=np.float32),
            np.zeros(128 * KC - V, dtype=np.float32)])
        bias_b.append(np.ascontiguousarray(bv.reshape(KC, 128).T))
    ones = np.ones((128, 72), dtype=np.float32)

    nc = _build_program(V)
    in_maps = []
    for core in range(NCORES):
        b, g = core // 2, core % 2
        in_maps.append({
            "xt": xt_b[b], "wqk": wqk_g[g], "wv": wv_g[g], "wo": wo_g[g],
            "biasv": bias_b[b], "onesd": ones,
        })

    trace = bool(os.environ.get("BASSK_TRACE"))
    if trace:
        _install_profile_hook()
    res = run_bass_kernel_spmd(nc, in_maps, list(range(NCORES)), trace=trace)
    global last_exec_time_ns, last_results
    last_exec_time_ns = res.exec_time_ns
    last_results = res.results

    out = np.zeros((B, N, D), dtype=np.float32)
    for b in range(B):
        yb = (res.results[2 * b]["y"].astype(np.float32)
              + res.results[2 * b + 1]["y"].astype(np.float32))
        out[b][idx[b]] = yb[:nv[b]]
    out += b_out
    return out


last_exec_time_ns = None
last_results = None


def _install_profile_hook():
    import types
    import antenv
    if 'antenv.axon_hooks' in sys.modules:
        return
    import trn_agent_boot.trn_boot as tb
    _hook = tb._ntff_profile_via_ctypes('/opt/axon/libaxon_pjrt.so')
    mod = types.ModuleType('antenv.axon_hooks')
    mod.get_axon_ntff_profile_hook = lambda: _hook
    mod.set_axon_ntff_profile_hook = lambda h: None
    sys.modules['antenv.axon_hooks'] = mod
    antenv.axon_hooks = mod
    bass_utils.upload_artifacts = lambda tmpdir: "local://skipped"
